# revision 1
# baseline (speedup 1.0000x reference)
"""Modulated 1x1 conv (ModConv) on 8 Trainium2 NeuronCores.

out[b,o,h,w] = sum_c (style[b,c] * weight[o,c]) * x[b,c,h,w]

Strategy: pure data parallel over the batch — 2 samples per core. Per
sample the kernel modulates the (pre-transposed) weight with the style
vector on DVE (cheap: [512,128] elements), then runs a K=512 contraction
as 4 PSUM-accumulated matmuls per 512-wide output tile. Matmul operands
use float32r (TF32-like PE path: full-rate rows vs 4 cycles/row for
fp32, ~1e-4 rel err), so the problem is HBM-bound (~21 MB/core at
~360 GB/s). x streams in as 1 MB [128, 2048] chunks alternating between
the SP and ACT HWDGE rings; outputs leave via the gpsimd SWDGE ring so
they never stall the input streams.
"""

import numpy as np

import concourse.bass as bass
import concourse.mybir as mybir
from concourse.bass_utils import run_bass_kernel_spmd
from concourse.tile import TileContext

B, CIN, COUT, H, W = 16, 512, 128, 64, 64
HW = H * W
N_CORES = 8
BPC = B // N_CORES  # samples per core
P = 128
KT = CIN // P  # k-tiles per contraction
NTILE = 512  # one PSUM bank of fp32
NT = HW // NTILE
NHALF = 2  # x chunks per k-tile (n-direction)
NCHUNK = HW // NHALF
FP32 = mybir.dt.float32
F32R = mybir.dt.float32r

# This container's walrus (public-SDK build) accepts at most one sync
# wait command per instruction; Tile's sem assignment attaches one wait
# per depended-on proc. Hoist the excess onto dedicated wait
# instructions (the same InstEventSemaphore a bass `wait_ge` emits)
# immediately before the over-subscribed instruction on its own engine.
MAX_WAITS_PER_INST = 1


def _split_sync_waits(nc: bass.Bass, limit: int = MAX_WAITS_PER_INST) -> int:
    n_split = 0
    for f in nc.m.functions:
        for bb in f.blocks:
            out = []
            for ins in bb.instructions:
                si = getattr(ins, "sync_info", None)
                if si is not None and si.on_wait and len(si.on_wait) > limit:
                    waits = list(si.on_wait)
                    for w in waits[:-limit]:
                        n_split += 1
                        es = mybir.InstEventSemaphore(
                            name=f"{ins.name}-ws{n_split}",
                            opcode="EventSemaphore",
                            engine=ins.engine,
                            sync_info=mybir.SyncInfo(on_wait=[w], on_update=[]),
                        )
                        nc.register_instruction(es, overwrite=True)
                        out.append(es)
                    si.on_wait = waits[-limit:]
                out.append(ins)
            bb.instructions[:] = out
    return n_split


def build_kernel(
    reps: int = 1,
    bench_mode: bool = False,
    nhalf: int = NHALF,
    x_bufs: int | None = None,
    psum_bufs: int = 4,
    skip_out: bool = False,
    skip_compute: bool = False,
    out_every: int | None = None,
    x_three_queues: bool = False,
    o_bufs: int = 2,
    pack: str = "packall16",  # "none"|"pack2"|"packall"|"packall8"|"packall16"
    out_hwdge: bool = False,
) -> bass.Bass:
    """reps>1 replicates the whole per-sample pipeline in-program (same
    inputs, outputs rewritten) — used only by the bench to measure
    steady-state per-iteration time with per-call overhead cancelled.
    bench_mode writes the big output to internal DRAM and exposes only a
    4-byte token output, so per-call tunnel traffic is negligible."""
    nchunk = HW // nhalf
    if x_bufs is None:
        # Exactly one slot of slack beyond 2 samples in flight, so the
        # HWDGE rings never stall on a slot release (packall 8 vs 9
        # measured ~55 vs ~35 us/iter; packall8 16 vs 17 similar; one
        # MORE slot regresses again in both cases).
        x_bufs = {
            "none": 2 * KT * nhalf,
            "pack2": 8,
            "packall": 9,
            "packall8": 17,
            "packall16": 33,
        }[pack]
    if out_every is None:
        out_every = NT // nhalf
    nc = bass.Bass()
    x = nc.dram_tensor("x", [BPC, CIN, HW], F32R, kind="ExternalInput")
    styleT = nc.dram_tensor("styleT", [CIN, BPC], FP32, kind="ExternalInput")
    wT = nc.dram_tensor("wT", [CIN, COUT], FP32, kind="ExternalInput")
    if bench_mode:
        out = nc.dram_tensor("out_scratch", [BPC, COUT, HW], FP32)
        token = nc.dram_tensor("token", [1, 1], FP32, kind="ExternalOutput")
    else:
        out = nc.dram_tensor("out", [BPC, COUT, HW], FP32, kind="ExternalOutput")
        token = None

    # The two HWDGE rings (SP + ACT) stream x in parallel.
    x_dma_engines = [nc.sync, nc.scalar]
    if x_three_queues:
        x_dma_engines = [nc.sync, nc.scalar, nc.gpsimd]

    with TileContext(nc) as tc:
        with (
            tc.tile_pool(name="consts", bufs=1) as cpool,
            tc.tile_pool(name="xs", bufs=x_bufs) as xpool,
            tc.tile_pool(name="os", bufs=o_bufs) as opool,
            tc.tile_pool(name="ps", bufs=psum_bufs, space="PSUM") as pspool,
        ):
            wT_sb = cpool.tile([P, KT, COUT], FP32)
            nc.sync.dma_start(out=wT_sb[:], in_=wT[:].rearrange("(t p) o -> p t o", p=P))
            sT_sb = cpool.tile([P, KT, BPC], FP32)
            nc.scalar.dma_start(
                out=sT_sb[:], in_=styleT[:].rearrange("(t p) b -> p t b", p=P)
            )
            # Per-sample modulated (transposed) weights: mw[p, b, t, o].
            # Stored as float32r so the PE takes the fast fp32 path.
            mw_sb = cpool.tile([P, BPC, KT, COUT], F32R)
            for b in range(BPC):
                for t in range(KT):
                    nc.vector.tensor_scalar_mul(
                        mw_sb[:, b, t, :], wT_sb[:, t, :], sT_sb[:, t, b : b + 1]
                    )

            dma_i = 0
            for _rep in range(reps):
                for b in range(BPC):
                    ntile, nt = NTILE, NT
                    if pack in ("packall", "packall8", "packall16"):
                        # One DMA per HW-chunk carrying all 4 k-tiles:
                        # packall: [128, 4, 1024] = 2 MB, feeds 2 n-tiles;
                        # packall8: [128, 4, 512] = 1 MB, feeds 1 n-tile;
                        # packall16: [128, 4, 256] = 512 KB, one 256-wide
                        # n-tile each (fp32r stays full-rate at N>=256).
                        qn = {"packall": 4, "packall8": 8, "packall16": 16}[pack]
                        qw = HW // qn
                        if pack == "packall16":
                            ntile, nt = 256, HW // 256
                        xq = []
                        for q in range(qn):
                            xt = xpool.tile([P, KT, qw], F32R, tag="xt")
                            eng = x_dma_engines[dma_i % len(x_dma_engines)]
                            dma_i += 1
                            eng.dma_start(
                                out=xt[:],
                                in_=x[b, :, q * qw : (q + 1) * qw].rearrange(
                                    "(t p) n -> p t n", p=P
                                ),
                            )
                            xq.append(xt)

                        def rhs(n, t, _ntile=ntile, _nper=None):
                            q, j = divmod(n, max(nt // qn, 1))
                            return xq[q][:, t, j * _ntile : (j + 1) * _ntile]
                    elif pack == "pack2":
                        # One DMA per (k-pair, HW-half): [128, 2, 2048] = 2 MB.
                        xg = [[None, None], [None, None]]
                        for h in range(2):
                            for g in range(2):
                                xt = xpool.tile([P, 2, HW // 2], F32R, tag="xt")
                                eng = x_dma_engines[dma_i % len(x_dma_engines)]
                                dma_i += 1
                                eng.dma_start(
                                    out=xt[:],
                                    in_=x[
                                        b,
                                        g * 2 * P : (g + 1) * 2 * P,
                                        h * (HW // 2) : (h + 1) * (HW // 2),
                                    ].rearrange("(t p) n -> p t n", p=P),
                                )
                                xg[g][h] = xt

                        def rhs(n, t):
                            h, j = divmod(n, NT // 2)
                            return xg[t // 2][h][
                                :, t % 2, j * NTILE : (j + 1) * NTILE
                            ]
                    else:
                        # x chunks: xh[t][h] = one (k-tile, HW-chunk) = 1 MB
                        xh = [[None] * nhalf for _ in range(KT)]
                        for h in range(nhalf):
                            for t in range(KT):
                                xt = xpool.tile([P, nchunk], F32R, tag="xt")
                                eng = x_dma_engines[dma_i % len(x_dma_engines)]
                                dma_i += 1
                                eng.dma_start(
                                    out=xt[:],
                                    in_=x[
                                        b,
                                        t * P : (t + 1) * P,
                                        h * nchunk : (h + 1) * nchunk,
                                    ],
                                )
                                xh[t][h] = xt

                        def rhs(n, t):
                            h, j = divmod(n, NT // nhalf)
                            return xh[t][h][:, j * NTILE : (j + 1) * NTILE]

                    if skip_compute:
                        continue
                    oev = out_every * (nt // NT)  # keep out-chunk BYTE size fixed
                    ot = opool.tile([P, HW], FP32, tag="ot")
                    for n in range(nt):
                        ps = pspool.tile([P, ntile], FP32, tag="ps")
                        for t in range(KT):
                            nc.tensor.matmul(
                                ps[:],
                                mw_sb[:, b, t, :],
                                rhs(n, t),
                                start=(t == 0),
                                stop=(t == KT - 1),
                            )
                        nc.vector.tensor_copy(
                            out=ot[:, n * ntile : (n + 1) * ntile], in_=ps[:]
                        )
                        if not skip_out and (n + 1) % oev == 0:
                            lo = (n + 1 - oev) * ntile
                            hi = (n + 1) * ntile
                            if out_hwdge:
                                oeng = x_dma_engines[dma_i % len(x_dma_engines)]
                                dma_i += 1
                            else:
                                oeng = nc.gpsimd
                            oeng.dma_start(out=out[b, :, lo:hi], in_=ot[:, lo:hi])
            if token is not None:
                nc.gpsimd.dma_start(out=token[:], in_=mw_sb[:1, 0, 0, :1])

    _split_sync_waits(nc)
    return nc


_NC_CACHE: bass.Bass | None = None


def _get_nc() -> bass.Bass:
    global _NC_CACHE
    if _NC_CACHE is None:
        _NC_CACHE = build_kernel()
    return _NC_CACHE


def make_in_maps(x: np.ndarray, style: np.ndarray, weight: np.ndarray):
    x_flat = np.ascontiguousarray(np.asarray(x, dtype=np.float32)).reshape(B, CIN, HW)
    styleT = np.ascontiguousarray(np.asarray(style, dtype=np.float32).T)  # [CIN, B]
    wT = np.ascontiguousarray(np.asarray(weight, dtype=np.float32).T)  # [CIN, COUT]
    in_maps = []
    for c in range(N_CORES):
        sl = slice(c * BPC, (c + 1) * BPC)
        in_maps.append(
            {
                "x": x_flat[sl],
                "styleT": np.ascontiguousarray(styleT[:, sl]),
                "wT": wT,
            }
        )
    return in_maps


def gather_out(results) -> np.ndarray:
    out = np.empty((B, COUT, H, W), dtype=np.float32)
    for c in range(N_CORES):
        out[c * BPC : (c + 1) * BPC] = results[c]["out"].reshape(BPC, COUT, H, W)
    return out


def kernel(x: np.ndarray, style: np.ndarray, weight: np.ndarray) -> np.ndarray:
    nc = _get_nc()
    in_maps = make_in_maps(x, style, weight)
    res = run_bass_kernel_spmd(nc, in_maps, core_ids=list(range(N_CORES)))
    return gather_out(res.results)



# revision 2
# speedup vs baseline: 2.2711x; 2.2711x over previous
"""Modulated 1x1 conv (ModConv) on 8 Trainium2 NeuronCores.

out[b,o,h,w] = sum_c (style[b,c] * weight[o,c]) * x[b,c,h,w]

Strategy: pure data parallel over the batch — 2 samples per core. The
kernel is HBM-bound (per-core aggregate DMA ~358 GB/s), so the main
lever is bytes on the wire: x is cast to fp16 on the host (the harness
gate is rel_err < 2e-2; fp16 end-to-end lands ~4e-4) and the output
leaves the device as fp16, halving traffic vs fp32 — ~10.5 MB/core
instead of ~21 MB. Per sample the kernel modulates the (pre-transposed)
weight with the style vector on DVE in fp32, rounds to fp16, then runs
a K=512 contraction as 4 PSUM-accumulated fp16 matmuls (full PE rate,
1 cycle/row) per 512-wide output tile. x streams in as [128, 4, qw]
fp16 chunks alternating between the SP and ACT HWDGE rings; outputs
are downcast to fp16 on DVE during the PSUM->SBUF copy and leave via
the gpsimd SWDGE ring so they never stall the input streams.
"""

import numpy as np

import concourse.bass as bass
import concourse.mybir as mybir
from concourse.bass_utils import run_bass_kernel_spmd
from concourse.tile import TileContext

B, CIN, COUT, H, W = 16, 512, 128, 64, 64
HW = H * W
N_CORES = 8
BPC = B // N_CORES  # samples per core
P = 128
KT = CIN // P  # k-tiles per contraction
FP32 = mybir.dt.float32
FP16 = mybir.dt.float16

# This container's walrus (public-SDK build) accepts at most one sync
# wait command per instruction; Tile's sem assignment attaches one wait
# per depended-on proc. Hoist the excess onto dedicated wait
# instructions (the same InstEventSemaphore a bass `wait_ge` emits)
# immediately before the over-subscribed instruction on its own engine.
MAX_WAITS_PER_INST = 1


def _split_sync_waits(nc: bass.Bass, limit: int = MAX_WAITS_PER_INST) -> int:
    n_split = 0
    for f in nc.m.functions:
        for bb in f.blocks:
            out = []
            for ins in bb.instructions:
                si = getattr(ins, "sync_info", None)
                if si is not None and si.on_wait and len(si.on_wait) > limit:
                    waits = list(si.on_wait)
                    for w in waits[:-limit]:
                        n_split += 1
                        es = mybir.InstEventSemaphore(
                            name=f"{ins.name}-ws{n_split}",
                            opcode="EventSemaphore",
                            engine=ins.engine,
                            sync_info=mybir.SyncInfo(on_wait=[w], on_update=[]),
                        )
                        nc.register_instruction(es, overwrite=True)
                        out.append(es)
                    si.on_wait = waits[-limit:]
                out.append(ins)
            bb.instructions[:] = out
    return n_split


def build_kernel(
    reps: int = 1,
    bench_mode: bool = False,
    qn: int = 8,  # x DMA chunks per sample, each [128, KT, HW/qn] fp16
    x_bufs: int | None = None,
    psum_bufs: int = 4,
    o_bufs: int = 2,
    out_chunks: int = 2,  # output DMAs per sample
    x_three_queues: bool = False,
) -> bass.Bass:
    """reps>1 replicates the whole per-sample pipeline in-program (same
    inputs, outputs rewritten) — used only by the bench to measure
    steady-state per-iteration time with per-call overhead cancelled.
    bench_mode writes the big output to internal DRAM and exposes only a
    4-byte token output, so per-call tunnel traffic is negligible."""
    qw = HW // qn
    ntile = min(512, qw)  # PSUM tile width (512 fp32 = one 2 KB bank)
    nt = HW // ntile
    if x_bufs is None:
        # One slot of slack beyond 2 samples in flight, so the HWDGE
        # rings never stall on a slot release (measured on the fp32
        # variant: +1 slot ~20 us faster; one more regresses again).
        x_bufs = 2 * qn + 1
    nc = bass.Bass()
    x = nc.dram_tensor("x", [BPC, CIN, HW], FP16, kind="ExternalInput")
    styleT = nc.dram_tensor("styleT", [CIN, BPC], FP32, kind="ExternalInput")
    wT = nc.dram_tensor("wT", [CIN, COUT], FP32, kind="ExternalInput")
    if bench_mode:
        out = nc.dram_tensor("out_scratch", [BPC, COUT, HW], FP16)
        token = nc.dram_tensor("token", [1, 1], FP16, kind="ExternalOutput")
    else:
        out = nc.dram_tensor("out", [BPC, COUT, HW], FP16, kind="ExternalOutput")
        token = None

    # The two HWDGE rings (SP + ACT) stream x in parallel.
    x_dma_engines = [nc.sync, nc.scalar]
    if x_three_queues:
        x_dma_engines = [nc.sync, nc.scalar, nc.gpsimd]

    with TileContext(nc) as tc:
        with (
            tc.tile_pool(name="consts", bufs=1) as cpool,
            tc.tile_pool(name="xs", bufs=x_bufs) as xpool,
            tc.tile_pool(name="os", bufs=o_bufs) as opool,
            tc.tile_pool(name="ps", bufs=psum_bufs, space="PSUM") as pspool,
        ):
            wT_sb = cpool.tile([P, KT, COUT], FP32)
            nc.sync.dma_start(out=wT_sb[:], in_=wT[:].rearrange("(t p) o -> p t o", p=P))
            sT_sb = cpool.tile([P, KT, BPC], FP32)
            nc.scalar.dma_start(
                out=sT_sb[:], in_=styleT[:].rearrange("(t p) b -> p t b", p=P)
            )
            # Per-sample modulated (transposed) weights: mw[p, b, t, o],
            # computed in fp32 on DVE, rounded to fp16 on the write.
            mw_sb = cpool.tile([P, BPC, KT, COUT], FP16)
            for b in range(BPC):
                for t in range(KT):
                    nc.vector.tensor_scalar_mul(
                        mw_sb[:, b, t, :], wT_sb[:, t, :], sT_sb[:, t, b : b + 1]
                    )

            oev = nt // out_chunks  # n-tiles per output DMA
            dma_i = 0
            for _rep in range(reps):
                for b in range(BPC):
                    # One DMA per HW-chunk carrying all 4 k-tiles.
                    xq = []
                    for q in range(qn):
                        xt = xpool.tile([P, KT, qw], FP16, tag="xt")
                        eng = x_dma_engines[dma_i % len(x_dma_engines)]
                        dma_i += 1
                        eng.dma_start(
                            out=xt[:],
                            in_=x[b, :, q * qw : (q + 1) * qw].rearrange(
                                "(t p) n -> p t n", p=P
                            ),
                        )
                        xq.append(xt)

                    ot = opool.tile([P, HW], FP16, tag="ot")
                    for n in range(nt):
                        ps = pspool.tile([P, ntile], FP32, tag="ps")
                        q, j = divmod(n, max(nt // qn, 1))
                        for t in range(KT):
                            nc.tensor.matmul(
                                ps[:],
                                mw_sb[:, b, t, :],
                                xq[q][:, t, j * ntile : (j + 1) * ntile],
                                start=(t == 0),
                                stop=(t == KT - 1),
                            )
                        nc.vector.tensor_copy(
                            out=ot[:, n * ntile : (n + 1) * ntile], in_=ps[:]
                        )
                        if (n + 1) % oev == 0:
                            lo = (n + 1 - oev) * ntile
                            hi = (n + 1) * ntile
                            nc.gpsimd.dma_start(out=out[b, :, lo:hi], in_=ot[:, lo:hi])
            if token is not None:
                nc.gpsimd.dma_start(out=token[:], in_=mw_sb[:1, 0, 0, :1])

    _split_sync_waits(nc)
    return nc


_NC_CACHE: bass.Bass | None = None


def _get_nc() -> bass.Bass:
    global _NC_CACHE
    if _NC_CACHE is None:
        _NC_CACHE = build_kernel()
    return _NC_CACHE


def make_in_maps(x: np.ndarray, style: np.ndarray, weight: np.ndarray):
    x_flat = np.asarray(x, dtype=np.float32).reshape(B, CIN, HW).astype(np.float16)
    styleT = np.ascontiguousarray(np.asarray(style, dtype=np.float32).T)  # [CIN, B]
    wT = np.ascontiguousarray(np.asarray(weight, dtype=np.float32).T)  # [CIN, COUT]
    in_maps = []
    for c in range(N_CORES):
        sl = slice(c * BPC, (c + 1) * BPC)
        in_maps.append(
            {
                "x": np.ascontiguousarray(x_flat[sl]),
                "styleT": np.ascontiguousarray(styleT[:, sl]),
                "wT": wT,
            }
        )
    return in_maps


def gather_out(results) -> np.ndarray:
    out = np.empty((B, COUT, H, W), dtype=np.float32)
    for c in range(N_CORES):
        out[c * BPC : (c + 1) * BPC] = (
            results[c]["out"].astype(np.float32).reshape(BPC, COUT, H, W)
        )
    return out


def kernel(x: np.ndarray, style: np.ndarray, weight: np.ndarray) -> np.ndarray:
    nc = _get_nc()
    in_maps = make_in_maps(x, style, weight)
    res = run_bass_kernel_spmd(nc, in_maps, core_ids=list(range(N_CORES)))
    return gather_out(res.results)


# revision 25
# speedup vs baseline: 3.3097x; 1.4574x over previous
"""Modulated 1x1 conv (ModConv) on 8 Trainium2 NeuronCores.

out[b,o,h,w] = sum_c (style[b,c] * weight[o,c]) * x[b,c,h,w]

Strategy: pure data parallel over the batch — 2 samples per core. The
kernel is DMA-bound, so two levers dominate:

1. Bytes on the wire. x is cast to fp16 on the host (the harness gate
   is rel_err < 2e-2; fp16 end-to-end lands ~4e-4) and the output
   leaves the device as fp16 — ~10.5 MB/core instead of ~21 MB fp32.
   The PE stays full-rate (1 cycle/row) for fp16, same as fp32r.
2. DMA queue parallelism. Per-queue throughput caps well below the
   aggregate (measured: 2 queues ~390 GB/s, 3 queues ~540 GB/s), so
   all three DMA-capable rings (SP + ACT HWDGE, Pool SWDGE) carry x
   AND output chunks round-robin, ~3.5 MB/ring. Output DMAs are
   deferred by one sample so their compute-completion waits are
   already satisfied at issue — without this, a compute-gated out DMA
   head-of-line-blocks later x chunks queued behind it on the same
   engine sequencer (measured up to 2x slowdown).

x is pre-transposed on the host to [qn, 128, KT, qw] per sample so
each chunk DMA reads one contiguous block (4 KB per-partition
descriptor lines). Per sample the kernel modulates the
(pre-transposed) weight with the style vector on DVE in fp32, rounds
to fp16, then runs the K=512 contraction as 4 PSUM-accumulated fp16
matmuls per 512-wide output tile; DVE downcasts PSUM->SBUF to fp16.

Per-core floor: PE 13.7 us busy, DMA ~10.5 MB across 3 rings. The
axon-tunnel slope bench draws 19.6-33 us for this config run-to-run
(device clock/p-state state between RPC calls); fp32 baseline was
60-62 us.
"""

import numpy as np

import concourse.bass as bass
import concourse.mybir as mybir
from concourse.bass_utils import run_bass_kernel_spmd
from concourse.tile import TileContext

B, CIN, COUT, H, W = 16, 512, 128, 64, 64
HW = H * W
N_CORES = 8
BPC = B // N_CORES  # samples per core
P = 128
KT = CIN // P  # k-tiles per contraction
FP32 = mybir.dt.float32
FP16 = mybir.dt.float16

# This container's walrus (public-SDK build) accepts at most one sync
# wait command per instruction; Tile's sem assignment attaches one wait
# per depended-on proc. Hoist the excess onto dedicated wait
# instructions (the same InstEventSemaphore a bass `wait_ge` emits)
# immediately before the over-subscribed instruction on its own engine.
MAX_WAITS_PER_INST = 1


def _split_sync_waits(nc: bass.Bass, limit: int = MAX_WAITS_PER_INST) -> int:
    n_split = 0
    for f in nc.m.functions:
        for bb in f.blocks:
            out = []
            for ins in bb.instructions:
                si = getattr(ins, "sync_info", None)
                if si is not None and si.on_wait and len(si.on_wait) > limit:
                    waits = list(si.on_wait)
                    for w in waits[:-limit]:
                        n_split += 1
                        es = mybir.InstEventSemaphore(
                            name=f"{ins.name}-ws{n_split}",
                            opcode="EventSemaphore",
                            engine=ins.engine,
                            sync_info=mybir.SyncInfo(on_wait=[w], on_update=[]),
                        )
                        nc.register_instruction(es, overwrite=True)
                        out.append(es)
                    si.on_wait = waits[-limit:]
                out.append(ins)
            bb.instructions[:] = out
    return n_split


def build_kernel(
    reps: int = 1,
    bench_mode: bool = False,
    qn: int = 8,  # x DMA chunks per sample, each [128, KT, HW/qn] fp16
    x_bufs: int | None = None,
    psum_bufs: int = 4,
    o_bufs: int = 4,
    out_chunks: int = 2,  # output DMAs per sample
    x_engines: str = "sag",  # s=SP, a=ACT (HWDGE), g=Pool (SWDGE)
    out_engines: str = "s",  # round-robin over these for output DMAs
    defer_out: int = 1,  # defer out DMAs this many samples; join x round-robin
    xlay: str = "c",  # x DRAM layout: "c"=pre-transposed contiguous chunks,
    #                   "r"=[CIN, HW] with a strided rearrange on the DMA
) -> bass.Bass:
    """reps>1 replicates the whole per-sample pipeline in-program (same
    inputs, outputs rewritten) — used only by the bench to measure
    steady-state per-iteration time with per-call overhead cancelled.
    bench_mode writes the big output to internal DRAM and exposes only a
    4-byte token output, so per-call tunnel traffic is negligible."""
    qw = HW // qn
    ntile = min(512, qw)  # PSUM tile width (512 fp32 = one 2 KB bank)
    nt = HW // ntile
    if x_bufs is None:
        # One slot of slack beyond 2 samples in flight, so the HWDGE
        # rings never stall on a slot release (measured on the fp32
        # variant: +1 slot ~20 us faster; one more regresses again).
        x_bufs = 2 * qn + 1
    nc = bass.Bass()
    # xlay="c": x arrives pre-transposed on the host to [BPC, qn, P, KT, qw]:
    # each chunk DMA reads one fully contiguous block with 4 KB+ per-partition
    # descriptor lines (vs 1 KB strided lines from a [CIN, HW] layout).
    if xlay == "c":
        x = nc.dram_tensor("x", [BPC, qn, P, KT, qw], FP16, kind="ExternalInput")
    else:
        x = nc.dram_tensor("x", [BPC, CIN, HW], FP16, kind="ExternalInput")
    styleT = nc.dram_tensor("styleT", [CIN, BPC], FP32, kind="ExternalInput")
    wT = nc.dram_tensor("wT", [CIN, COUT], FP32, kind="ExternalInput")
    if bench_mode:
        out = nc.dram_tensor("out_scratch", [BPC, COUT, HW], FP16)
        token = nc.dram_tensor("token", [1, 1], FP16, kind="ExternalOutput")
    else:
        out = nc.dram_tensor("out", [BPC, COUT, HW], FP16, kind="ExternalOutput")
        token = None

    # The HWDGE rings (SP + ACT by default) stream x in parallel.
    eng_map = {"s": nc.sync, "a": nc.scalar, "g": nc.gpsimd}
    x_dma_engines = [eng_map[c] for c in x_engines]
    out_dma_engines = [eng_map[c] for c in out_engines]

    with TileContext(nc) as tc:
        with (
            tc.tile_pool(name="consts", bufs=1) as cpool,
            tc.tile_pool(name="xs", bufs=x_bufs) as xpool,
            tc.tile_pool(name="os", bufs=o_bufs) as opool,
            tc.tile_pool(name="ps", bufs=psum_bufs, space="PSUM") as pspool,
        ):
            wT_sb = cpool.tile([P, KT, COUT], FP32)
            nc.sync.dma_start(out=wT_sb[:], in_=wT[:].rearrange("(t p) o -> p t o", p=P))
            sT_sb = cpool.tile([P, KT, BPC], FP32)
            nc.scalar.dma_start(
                out=sT_sb[:], in_=styleT[:].rearrange("(t p) b -> p t b", p=P)
            )
            # Per-sample modulated (transposed) weights: mw[p, b, t, o],
            # computed in fp32 on DVE, rounded to fp16 on the write.
            mw_sb = cpool.tile([P, BPC, KT, COUT], FP16)
            for b in range(BPC):
                for t in range(KT):
                    nc.vector.tensor_scalar_mul(
                        mw_sb[:, b, t, :], wT_sb[:, t, :], sT_sb[:, t, b : b + 1]
                    )

            oev = nt // out_chunks  # n-tiles per output DMA
            dma_i = 0
            out_i = 0
            pending = []  # deferred out DMAs: (b, lo, hi, ot)
            for _rep in range(reps):
                for b in range(BPC):
                    # Drain deferred out DMAs (their producing copies are
                    # >=defer_out samples old, so the sem wait is already
                    # satisfied — no head-of-line blocking on the ring),
                    # interleaved round-robin with this sample's x chunks.
                    issue_now = []
                    if defer_out and len(pending) > defer_out * out_chunks:
                        issue_now = pending[: len(pending) - defer_out * out_chunks]
                        pending = pending[len(pending) - defer_out * out_chunks :]
                    # One DMA per HW-chunk carrying all 4 k-tiles.
                    xq = []
                    for q in range(qn):
                        if issue_now and q % 2 == 0:
                            ob, lo, hi, oot = issue_now.pop(0)
                            eng = x_dma_engines[dma_i % len(x_dma_engines)]
                            dma_i += 1
                            eng.dma_start(out=out[ob, :, lo:hi], in_=oot[:, lo:hi])
                        xt = xpool.tile([P, KT, qw], FP16, tag="xt")
                        eng = x_dma_engines[dma_i % len(x_dma_engines)]
                        dma_i += 1
                        if xlay == "c":
                            eng.dma_start(out=xt[:], in_=x[b, q])
                        else:
                            eng.dma_start(
                                out=xt[:],
                                in_=x[b, :, q * qw : (q + 1) * qw].rearrange(
                                    "(t p) n -> p t n", p=P
                                ),
                            )
                        xq.append(xt)
                    for ob, lo, hi, oot in issue_now:
                        eng = x_dma_engines[dma_i % len(x_dma_engines)]
                        dma_i += 1
                        eng.dma_start(out=out[ob, :, lo:hi], in_=oot[:, lo:hi])

                    ot = opool.tile([P, HW], FP16, tag="ot")
                    for n in range(nt):
                        ps = pspool.tile([P, ntile], FP32, tag="ps")
                        q, j = divmod(n, max(nt // qn, 1))
                        for t in range(KT):
                            nc.tensor.matmul(
                                ps[:],
                                mw_sb[:, b, t, :],
                                xq[q][:, t, j * ntile : (j + 1) * ntile],
                                start=(t == 0),
                                stop=(t == KT - 1),
                            )
                        nc.vector.tensor_copy(
                            out=ot[:, n * ntile : (n + 1) * ntile], in_=ps[:]
                        )
                        if (n + 1) % oev == 0:
                            lo = (n + 1 - oev) * ntile
                            hi = (n + 1) * ntile
                            if defer_out:
                                pending.append((b, lo, hi, ot))
                            else:
                                oeng = out_dma_engines[out_i % len(out_dma_engines)]
                                out_i += 1
                                oeng.dma_start(out=out[b, :, lo:hi], in_=ot[:, lo:hi])
            for ob, lo, hi, oot in pending:
                eng = x_dma_engines[dma_i % len(x_dma_engines)]
                dma_i += 1
                eng.dma_start(out=out[ob, :, lo:hi], in_=oot[:, lo:hi])
            if token is not None:
                nc.gpsimd.dma_start(out=token[:], in_=mw_sb[:1, 0, 0, :1])

    _split_sync_waits(nc)
    return nc


_NC_CACHE: bass.Bass | None = None


def _get_nc() -> bass.Bass:
    global _NC_CACHE
    if _NC_CACHE is None:
        _NC_CACHE = build_kernel()
    return _NC_CACHE


def make_in_maps(
    x: np.ndarray, style: np.ndarray, weight: np.ndarray, qn: int = 8, xlay: str = "c"
):
    qw = HW // qn
    # xlay="c": [B, CIN, HW] -> fp16 [B, qn, P, KT, qw]: chunk q / partition
    # p / k-tile t holds x[b, t*P + p, q*qw : (q+1)*qw] — the layout each
    # chunk DMA consumes as one contiguous block.
    if xlay == "c":
        x_t = (
            np.asarray(x, dtype=np.float32)
            .reshape(B, KT, P, qn, qw)
            .transpose(0, 3, 2, 1, 4)
            .astype(np.float16)
        )
    else:
        x_t = np.asarray(x, dtype=np.float32).reshape(B, CIN, HW).astype(np.float16)
    styleT = np.ascontiguousarray(np.asarray(style, dtype=np.float32).T)  # [CIN, B]
    wT = np.ascontiguousarray(np.asarray(weight, dtype=np.float32).T)  # [CIN, COUT]
    in_maps = []
    for c in range(N_CORES):
        sl = slice(c * BPC, (c + 1) * BPC)
        in_maps.append(
            {
                "x": np.ascontiguousarray(x_t[sl]),
                "styleT": np.ascontiguousarray(styleT[:, sl]),
                "wT": wT,
            }
        )
    return in_maps


def gather_out(results) -> np.ndarray:
    out = np.empty((B, COUT, H, W), dtype=np.float32)
    for c in range(N_CORES):
        out[c * BPC : (c + 1) * BPC] = (
            results[c]["out"].astype(np.float32).reshape(BPC, COUT, H, W)
        )
    return out


def kernel(x: np.ndarray, style: np.ndarray, weight: np.ndarray) -> np.ndarray:
    nc = _get_nc()
    in_maps = make_in_maps(x, style, weight)
    res = run_bass_kernel_spmd(nc, in_maps, core_ids=list(range(N_CORES)))
    return gather_out(res.results)
